# revision 1
# baseline (speedup 1.0000x reference)
"""DRSformer sparse channel-attention block on 8 Trainium2 cores.

Sharding: the 128 image rows are split across 8 cores (16 rows each, 1-row
zero-padded halo). Per core: qkv 1x1-conv and the depthwise 3x3 conv run in
float32r (TF32-class) on TensorE — the depthwise conv as diagonal-stationary
matmuls PSUM-accumulated over the 9 taps on a 1-column-padded input so every
tap streams a flat 512-wide chunk; the two image-edge columns per row are
recomputed exactly on VectorE afterwards. q/k are split hi/lo into two bf16
planes (hi + residual) and DMA-transposed to [n, c] layout; per-head gram
matmuls (hi*hi + hi*lo + lo*hi) recover near-fp32 attention logits AND the
q/k l2-norms in one pass, contracting over the core's pixels. A 295KB
AllReduce combines partial grams across cores. Exact top-k selection uses a
rank matrix (all-pairs compare + row-sum); the four top-k softmaxes collapse
into one effective matrix P = E * sum_k (a_k/S_k)*[rank<=k], so all four
attention applications become a single P @ v matmul (bf16). Dense projection,
and the row-sharded output is gathered on host.
"""
import sys
for _p in ('/opt/trn_rl_repo', '/root/.axon_site/_ro/trn_rl_repo'):
    if _p not in sys.path:
        sys.path.insert(0, _p)

import numpy as np
import ml_dtypes

import concourse.bass as bass
import concourse.tile as tile
from concourse.tile import add_dep_helper
from concourse import mybir
from concourse import bass_utils
from concourse.masks import make_identity

f32 = mybir.dt.float32
f32r = mybir.dt.float32r
bf16 = mybir.dt.bfloat16
AF = mybir.ActivationFunctionType
OP = mybir.AluOpType

B, DIM, HEADS, HH, WW = 2, 384, 8, 128, 128
C = DIM // HEADS            # 48
NCORES = 8
RPC = HH // NCORES          # 16 rows per core
NPX = RPC * WW              # 2048 local pixels per batch
NPXH = (RPC + 2) * WW       # 2304 with halo rows
NCH = NPX // 128            # 16 n-chunks of 128
KVALS = [C // 2, C * 2 // 3, C * 3 // 4, C * 4 // 5]   # 24, 32, 36, 38
TAPS = [(0, 0), (-1, -1), (-1, 1), (1, -1), (1, 1), (0, -1), (0, 1), (-1, 0), (1, 0)]


def _ct_runs(h):
    """Head h's 48 channels as runs over 128-wide channel tiles:
    (ct, lo, n, c_off)."""
    out = []
    g0, c = h * C, 0
    while c < C:
        t, r = (g0 + c) // 128, (g0 + c) % 128
        n = min(C - c, 128 - r)
        out.append((t, r, n, c))
        c += n
    return out


def _build_bass():
    nc = bass.Bass("TRN2", target_bir_lowering=False, num_devices=NCORES)

    x_sh = nc.dram_tensor("x_sh", [B, 3, 128, NPXH], f32r, kind="ExternalInput").ap()
    wqkvT = nc.dram_tensor("wqkvT", [3, 128, 1152], f32r, kind="ExternalInput").ap()
    diagw = nc.dram_tensor("diagw", [9, 9, 128, 128], f32r, kind="ExternalInput").ap()
    wprojPT = nc.dram_tensor("wprojPT", [4, 96, 384], bf16, kind="ExternalInput").ap()
    taucol = nc.dram_tensor("taucol", [128, 4], f32, kind="ExternalInput").ap()
    acoefs = nc.dram_tensor("acoefs", [128, 4], f32, kind="ExternalInput").ap()
    wcols = nc.dram_tensor("wcols", [9, 9, 128], f32, kind="ExternalInput").ap()
    out_sh = nc.dram_tensor("out_sh", [B, 3, 128, NPX], f32, kind="ExternalOutput").ap()

    with tile.TileContext(nc) as tc:
        _build_body(nc, tc, x_sh, wqkvT, diagw, wprojPT, taucol, acoefs, wcols, out_sh)

    _split_excess_waits(nc)
    return nc


def _build_body(nc, tc, x_sh, wqkvT, diagw, wprojPT, taucol, acoefs, wcols, out_sh):
    import contextlib
    ctx = contextlib.ExitStack()
    consts = ctx.enter_context(tc.tile_pool(name="consts", bufs=1))
    xp = ctx.enter_context(tc.tile_pool(name="xp", bufs=1))      # 3 tags
    qkvp = ctx.enter_context(tc.tile_pool(name="qkvp", bufs=2))  # 1 tag
    cqp = ctx.enter_context(tc.tile_pool(name="cqp", bufs=1))    # 1 tag (hi/lo)
    cvp = ctx.enter_context(tc.tile_pool(name="cvp", bufs=1))    # 3 tags
    qkRp = ctx.enter_context(tc.tile_pool(name="qkRp", bufs=8))  # 1 tag
    qkTp = ctx.enter_context(tc.tile_pool(name="qkTp", bufs=4))  # 1 tag
    gramp = ctx.enter_context(tc.tile_pool(name="gramp", bufs=2))
    smallp = ctx.enter_context(tc.tile_pool(name="smallp", bufs=2))
    cmpp = ctx.enter_context(tc.tile_pool(name="cmpp", bufs=1))
    pTp = ctx.enter_context(tc.tile_pool(name="pTp", bufs=2))
    pvp = ctx.enter_context(tc.tile_pool(name="pvp", bufs=1))    # 4 tags
    outp = ctx.enter_context(tc.tile_pool(name="outp", bufs=2))
    dramp = ctx.enter_context(tc.tile_pool(name="dramp", bufs=2, space="DRAM"))
    psmm = ctx.enter_context(tc.tile_pool(name="psmm", bufs=4, space="PSUM"))
    psgram = ctx.enter_context(tc.tile_pool(name="psgram", bufs=2, space="PSUM"))
    pspT = ctx.enter_context(tc.tile_pool(name="pspT", bufs=2, space="PSUM"))

    # ---- constants ----
    wqkv_sb = consts.tile([128, 3, 1152], f32r)
    nc.sync.dma_start(wqkv_sb, wqkvT.rearrange("k p o -> p k o"))
    diag_sb = consts.tile([128, 9, 9, 128], f32r)
    nc.sync.dma_start(diag_sb, diagw.rearrange("t c p f -> p t c f"))
    wproj_sb = consts.tile([96, 4, 384], bf16)
    nc.sync.dma_start(wproj_sb, wprojPT.rearrange("g p o -> p g o"))
    tau_sb = consts.tile([128, 4], f32)
    nc.sync.dma_start(tau_sb, taucol)
    ac_sb = consts.tile([128, 4], f32)
    nc.sync.dma_start(ac_sb, acoefs)
    wcol_sb = consts.tile([128, 9, 9], f32)
    nc.sync.dma_start(wcol_sb, wcols.rearrange("t c p -> p t c"))
    ident = consts.tile([128, 128], f32)
    make_identity(nc, ident)

    evict_flip = [0]
    last_evict = [None]

    def evict(dst, src):
        if evict_flip[0] % 2 == 0:
            e = nc.scalar.copy(dst, src)
        else:
            e = nc.vector.tensor_copy(dst, src)
        evict_flip[0] += 1
        last_evict[0] = e.ins
        return e

    prev_cc = [None]
    prev_gram_dma = [None]
    for b in range(B):
        # ---- load x ----
        x_t = []
        x_dma0 = [None]
        for kt in range(3):
            t = xp.tile([128, NPXH], f32r, tag=f"x{kt}", name=f"x_{b}_{kt}")
            d = nc.sync.dma_start(t, x_sh[b, kt])
            if prev_cc[0] is not None:
                # order next batch's x loads after the previous batch's LAST
                # gram DMA (not the collective): avoids SP queue head-of-line
                # deadlock while letting b1 compute overlap b0's AllReduce
                add_dep_helper(d.ins, prev_gram_dma[0], reason="batch gate x")
            if x_dma0[0] is None:
                x_dma0[0] = d.ins
            x_t.append(t)

        def edge_chain(dst_col, x0, ct, qt):
            """Exact conv for an image-edge column (16 rows, stride 128)."""
            first = True
            for ti, (dy, dx) in enumerate(TAPS):
                if (x0 == 0 and dx < 0) or (x0 == 127 and dx > 0):
                    continue
                soff = 1 + (1 + dy) * 128 + x0 + dx
                sap = bass.AP(tensor=qt.tensor, offset=qt.offset + soff,
                              ap=[qt.ap[0], [128, RPC], [1, 1]])
                wc = wcol_sb[:, ti, ct:ct + 1]
                if first:
                    nc.vector.tensor_scalar(out=dst_col, in0=sap, scalar1=wc,
                                            scalar2=None, op0=OP.mult)
                    first = False
                else:
                    nc.vector.scalar_tensor_tensor(out=dst_col, in0=sap, scalar=wc,
                                                   in1=dst_col, op0=OP.mult, op1=OP.add)

        def qkv_conv(ct, hilo):
            """qkv projection + depthwise conv for one 128-channel tile.
            hilo=True: [128, 2, NPX] bf16 (hi plane + residual lo);
            else [128, NPX] bf16."""
            qt = qkvp.tile([128, NPXH + 2], f32r, tag="qkv", name=f"qkv_{b}_{ct}")
            m1 = nc.gpsimd.memset(qt[:, 0:1].bitcast(f32), 0.0)
            m2 = nc.gpsimd.memset(qt[:, NPXH + 1:NPXH + 2].bitcast(f32), 0.0)
            add_dep_helper(m1.ins, x_dma0[0], reason="batch gate qt pad")
            add_dep_helper(m2.ins, x_dma0[0], reason="batch gate qt pad")
            for ch0 in range(0, NPXH, 512):
                cw = min(512, NPXH - ch0)
                ps = psmm.tile([128, 512], f32, tag="mm", name="psq")
                for kt in range(3):
                    nc.tensor.matmul(
                        ps[:, :cw],
                        lhsT=wqkv_sb[:, kt, ct * 128:(ct + 1) * 128],
                        rhs=x_t[kt][:, ch0:ch0 + cw],
                        start=(kt == 0), stop=(kt == 2),
                    )
                evict(qt[:, 1 + ch0:1 + ch0 + cw], ps[:, :cw])
            if hilo:
                co = cqp.tile([128, 2, NPX], bf16, tag="cq", name=f"co_{b}_{ct}")
                hi_v = co[:, 0, :]
                lo_v = co[:, 1, :]
            else:
                co = cvp.tile([128, NPX], bf16, tag=f"cv{ct - 6}", name=f"co_{b}_{ct}")
                hi_v = co
                lo_v = None
            for ch in range(4):
                ps = psmm.tile([128, 512], f32, tag="mm", name="psc")
                for ti, (dy, dx) in enumerate(TAPS):
                    off = 129 + ch * 512 + dy * 128 + dx
                    nc.tensor.matmul(
                        ps, lhsT=diag_sb[:, ti, ct, :], rhs=qt[:, off:off + 512],
                        start=(ti == 0), stop=(ti == len(TAPS) - 1),
                    )
                sl = slice(ch * 512, (ch + 1) * 512)
                evict(hi_v[:, sl], ps)
                if hilo:
                    nc.vector.tensor_tensor(out=lo_v[:, sl], in0=ps,
                                            in1=hi_v[:, sl], op=OP.subtract)
            # exact edge-column fixup on the hi plane; zero the lo edges
            for x0 in (0, 127):
                hc = hi_v.rearrange("p (r w) -> p r w", w=128)[:, :, x0:x0 + 1]
                edge_chain(hc, x0, ct, qt)
                if hilo:
                    lc = lo_v.rearrange("p (r w) -> p r w", w=128)[:, :, x0:x0 + 1]
                    nc.vector.memset(lc, 0.0)
            return co

        # ---- q/k: qkv+conv -> hi/lo transpose -> per-head repack+gram ----
        # repack runs are issued per source raw tile so raws release early
        qkT_tiles = {}
        gram_dmas = []
        ar_in = dramp.tile([HEADS, 96, 96], f32, tag="arin", name=f"arin{b}")

        def get_qkT(h):
            if h not in qkT_tiles:
                qkT_tiles[h] = qkTp.tile([128, NCH, 4, 48], bf16, tag="qkT",
                                         name=f"qkT_{b}_{h}")
            return qkT_tiles[h]

        def gram(h):
            qkT = qkT_tiles[h]
            # region A (cols 0:96) accumulates hi.hi + lo.hi; region B
            # (96:192) accumulates hi.lo; summed at eviction. Folding hi.hi
            # and hi.lo into one FD=192 matmul halves PE dispatch count.
            gps = psgram.tile([96, 192], f32, tag="gram", name="gps")
            for t in range(NCH):
                hi = qkT[:, t, 0:2, :]
                lo = qkT[:, t, 2:4, :]
                both = qkT[:, t, :, :]
                if t < NCH - 1:
                    nc.tensor.matmul(gps, lhsT=hi, rhs=both,
                                     start=(t == 0), stop=False)
                    nc.tensor.matmul(gps[:, 0:96], lhsT=lo, rhs=hi,
                                     start=False, stop=False)
                else:
                    nc.tensor.matmul(gps[:, 0:96], lhsT=lo, rhs=hi,
                                     start=False, stop=False)
                    nc.tensor.matmul(gps, lhsT=hi, rhs=both,
                                     start=False, stop=True)
            gsb = gramp.tile([96, 96], f32, tag="gsb", name="gsb")
            evict(gsb, gps[:, 0:96])
            nc.vector.tensor_add(gsb, gsb, gps[:, 96:192])
            gd = nc.sync.dma_start(ar_in[h], gsb)
            gram_dmas.append(gd.ins)

        # HW-DGE completion under-synchronization: a consumer released by a
        # wide DmaTransposeAnt's first queue-completion can read data still
        # in flight on the DMA's other fanned-out queues. Work around it by
        # deferring each round's repack copies until the NEXT round's
        # transposes exist, and gating them on those (one full conv round of
        # slack), so the wide transposes have long drained before any read.
        pending = {r: [] for r in range(3)}      # round -> [(dst, src)]
        tr_insts = {r: [] for r in range(3)}

        def flush_round(rnd, gates):
            for dst, srcslice in pending[rnd]:
                e = evict(dst, srcslice)
                for g in gates:
                    add_dep_helper(e.ins, g, reason="transpose drain slack")
            pending[rnd].clear()
            for h in range(HEADS):
                if max(t for (t, _, _, _) in _ct_runs(h)) == rnd:
                    gram(h)

        for pair_ct in range(3):
            for qk in range(2):
                ct = qk * 3 + pair_ct
                co = qkv_conv(ct, hilo=True)
                for pl in range(2):
                    tr = qkRp.tile([128, NCH, 128], bf16, tag="qkr",
                                   name=f"qkr_{b}_{ct}_{pl}")
                    # transposes isolated on the Activation DGE queues:
                    # concurrent plain copies on the same queues corrupt
                    # xbar-mode transposes (known HW hazard, untracked here)
                    td = nc.scalar.dma_start_transpose(tr, co[:, pl, :])
                    tr_insts[pair_ct].append(td.ins)
                    # planes in qkT: [q_hi | k_hi | q_lo | k_lo]
                    for h in range(HEADS):
                        for (t, r, n, c) in _ct_runs(h):
                            if t == pair_ct:
                                pending[pair_ct].append(
                                    (get_qkT(h)[:, :, 2 * pl + qk, c:c + n],
                                     tr[:, :, r:r + n]))
            if pair_ct > 0:
                flush_round(pair_ct - 1, tr_insts[pair_ct])

        # ---- v ----
        cv_t = []
        for ct in range(6, 9):
            cv_t.append(qkv_conv(ct, hilo=False))
        flush_round(2, [last_evict[0]])

        # ---- AllReduce partial grams ----
        ar_out = dramp.tile([HEADS, 96, 96], f32, tag="arout", name=f"arout{b}")
        cc = nc.gpsimd.collective_compute(
            "AllReduce", OP.add,
            replica_groups=[list(range(NCORES))],
            ins=[ar_in[:].opt()], outs=[ar_out[:].opt()],
        )
        for gd in gram_dmas:
            # explicit sem deps: the collective must not read ar_in before
            # every gram DMA has landed (Tile's transitive-clock reasoning
            # proved unsound for this on HW)
            add_dep_helper(cc.ins, gd, reason="cc waits gram dmas")
        prev_cc[0] = cc.ins
        prev_gram_dma[0] = gram_dmas[-1]

        # ---- post-AllReduce: dense tiles, 2 heads per tile at 64-row pitch ----
        arf = ar_out.rearrange("h i j -> (h i j)")
        kdiag = smallp.tile([HEADS, 48], f32, tag="kdiag", name="kdiag")
        for h in range(HEADS):
            base = h * 96 * 96 + 48 * 96 + 48
            src = bass.AP(tensor=arf.tensor, offset=arf.offset + base,
                          ap=[[0, 1], [97, 48]])
            _d = nc.sync.dma_start(kdiag[h:h + 1, :], src)
            add_dep_helper(_d.ins, cc.ins, reason="post-AR read after cc")
        kdd = dramp.tile([HEADS, 48], f32, tag="kdd", name=f"kdd{b}")
        nc.sync.dma_start(kdd, kdiag)

        pv_t = []
        for dt in range(4):
            at = smallp.tile([128, 48], f32, tag="attn", name="at")
            rq = smallp.tile([128, 1], f32, tag="rq", name="rq")
            rk = smallp.tile([128, 48], f32, tag="rk", name="rk")
            for _t in (at, rq, rk):
                _m = nc.gpsimd.memset(_t, 1.0)
                add_dep_helper(_m.ins, prev_cc[0], reason="post-AR gate")
            for e in range(2):
                h = 2 * dt + e
                r = 64 * e
                base = h * 96 * 96
                src = bass.AP(tensor=arf.tensor, offset=arf.offset + base + 48,
                              ap=[[96, 48], [1, 48]])
                _d1 = nc.sync.dma_start(at[r:r + 48, :], src)
                add_dep_helper(_d1.ins, cc.ins, reason="post-AR read after cc")
                srcq = bass.AP(tensor=arf.tensor, offset=arf.offset + base,
                               ap=[[97, 48], [1, 1]])
                _d2 = nc.sync.dma_start(rq[r:r + 48, :], srcq)
                add_dep_helper(_d2.ins, cc.ins, reason="post-AR read after cc")
                nc.sync.dma_start(rk[r:r + 48, :],
                                  kdd[h:h + 1, :].broadcast_to((48, 48)))

            # ---- normalize, rank, blended masked softmax ----
            nc.vector.reciprocal(rq, rq)
            nc.scalar.sqrt(rq, rq)
            nc.vector.reciprocal(rk, rk)
            nc.scalar.sqrt(rk, rk)
            an = smallp.tile([128, 48], f32, tag="an", name="an")
            nc.vector.tensor_scalar(out=an, in0=at, scalar1=rq,
                                    scalar2=None, op0=OP.mult)
            nc.vector.tensor_mul(an, an, rk)
            rank = smallp.tile([128, 48], f32, tag="rank", name="rank")
            for half in range(2):
                cmp = cmpp.tile([128, 24, 48], bf16, tag="cmp", name="cmp")
                io = half * 24
                in_j = bass.AP(tensor=an.tensor, offset=an.offset,
                               ap=[an.ap[0], [0, 24], [1, 48]])
                in_i = bass.AP(tensor=an.tensor, offset=an.offset + io,
                               ap=[an.ap[0], [1, 24], [0, 48]])
                nc.vector.tensor_tensor(out=cmp, in0=in_j, in1=in_i, op=OP.is_ge)
                nc.vector.tensor_reduce(out=rank[:, io:io + 24], in_=cmp,
                                        axis=mybir.AxisListType.X, op=OP.add)
            E = smallp.tile([128, 48], f32, tag="E", name="E")
            nc.scalar.activation(E, an, AF.Exp, scale=tau_sb[:, dt:dt + 1])
            W = smallp.tile([128, 48], f32, tag="W", name="W")
            junk = smallp.tile([128, 48], f32, tag="junk", name="junk")
            S = smallp.tile([128, 1], f32, tag="S", name="S")
            wcolv = smallp.tile([128, 1], f32, tag="wcolv", name="wcolv")
            for ki, kk in enumerate(KVALS):
                mk = smallp.tile([128, 48], bf16, tag="mk", name="mk")
                nc.vector.tensor_scalar(out=mk, in0=rank, scalar1=float(kk),
                                        scalar2=None, op0=OP.is_le)
                nc.vector.tensor_mul(junk, E, mk)
                nc.vector.tensor_reduce(out=S, in_=junk,
                                        axis=mybir.AxisListType.X, op=OP.add)
                nc.vector.reciprocal(S, S)
                nc.vector.tensor_mul(wcolv, S, ac_sb[:, ki:ki + 1])
                if ki == 0:
                    nc.vector.tensor_scalar(out=W, in0=mk, scalar1=wcolv,
                                            scalar2=None, op0=OP.mult)
                else:
                    nc.vector.scalar_tensor_tensor(out=W, in0=mk, scalar=wcolv,
                                                   in1=W, op0=OP.mult, op1=OP.add)
            P = smallp.tile([128, 48], f32, tag="P", name="P")
            nc.vector.tensor_mul(P, E, W)

            # ---- P^T pieces into v-aligned pair stationaries ----
            pair = dt
            pT = {}
            for e in range(2):
                for (vt, k0, nd, d0) in _ct_runs(2 * pair + e):
                    if (pair, vt) not in pT:
                        t = pTp.tile([128, 96], bf16, tag="pT", name=f"pT{pair}_{vt}")
                        _m = nc.vector.memset(t, 0.0)
                        add_dep_helper(_m.ins, prev_cc[0], reason="post-AR gate")
                        pT[(pair, vt)] = t
            for e in range(2):
                h = 2 * pair + e
                r = 64 * e
                tps = pspT.tile([48, 48], f32, tag="tps", name="tps")
                nc.tensor.transpose(tps, P[r:r + 48, :], ident[r:r + 48, r:r + 48])
                piece = smallp.tile([48, 48], bf16, tag="piece", name="piece")
                evict(piece, tps)
                for (vt, k0, nd, d0) in _ct_runs(h):
                    nc.sync.dma_start(
                        pT[(pair, vt)][k0:k0 + nd, e * 48: e * 48 + 48],
                        piece[d0:d0 + nd, :])

            # ---- P @ v for this pair ----
            pvt = pvp.tile([96, NPX], bf16, tag=f"pv{pair}", name=f"pv_{b}_{pair}")
            vts = sorted({vt for e in range(2)
                          for (vt, _, _, _) in _ct_runs(2 * pair + e)})
            for ch in range(4):
                ps = psmm.tile([128, 512], f32, tag="mm", name="pspv")
                for vi, vt in enumerate(vts):
                    nc.tensor.matmul(ps[:96, :], lhsT=pT[(pair, vt)],
                                     rhs=cv_t[vt][:, ch * 512:(ch + 1) * 512],
                                     start=(vi == 0), stop=(vi == len(vts) - 1))
                evict(pvt[:, ch * 512:(ch + 1) * 512], ps[:96, :])
            pv_t.append(pvt)

        # ---- out = Wproj @ pv ----
        for ot in range(3):
            for ch in range(4):
                ps = psmm.tile([128, 512], f32, tag="mm", name="pso")
                for p in range(4):
                    nc.tensor.matmul(ps, lhsT=wproj_sb[:, p, ot * 128:(ot + 1) * 128],
                                     rhs=pv_t[p][:, ch * 512:(ch + 1) * 512],
                                     start=(p == 0), stop=(p == 3))
                ot_sb = outp.tile([128, 512], f32, tag="osb", name="osb")
                evict(ot_sb, ps)
                nc.sync.dma_start(out_sh[b, ot, :, ch * 512:(ch + 1) * 512], ot_sb)

    ctx.close()


def _split_excess_waits(nc, cap=1):
    """walrus allows 1 sync-wait per instruction; Tile's tail drain can carry
    more — split extras into single-wait drains."""
    n_new = 0
    for fn in nc.m.functions:
        for bb in fn.blocks:
            insts = bb.instructions
            i = 0
            while i < len(insts):
                inst = insts[i]
                si = inst.sync_info
                if si is not None and len(si.on_wait) > cap:
                    waits = list(si.on_wait)
                    extras, keep = waits[:-cap], waits[-cap:]
                    inst.sync_info = mybir.SyncInfo(on_wait=keep,
                                                    on_update=list(si.on_update))
                    for w in extras:
                        d = mybir.InstDrain(name=f"{inst.name}-sw{n_new}",
                                            ins=[], outs=[])
                        d.engine = inst.engine
                        d.sync_info = mybir.SyncInfo(on_wait=[w], on_update=[])
                        nc.register_instruction(d, overwrite=True)
                        insts.insert(i, d)
                        i += 1
                        n_new += 1
                i += 1
    return n_new


_NC_CACHE = {}


def _get_nc():
    if "nc" not in _NC_CACHE:
        _NC_CACHE["nc"] = _build_bass()
    return _NC_CACHE["nc"]


def _prep_inputs(x, w_qkv, w_dw, w_proj, temperature, avals):
    xpad = np.zeros((B, DIM, HH + 2, WW), np.float32)
    xpad[:, :, 1:HH + 1] = x
    wqkvT = np.ascontiguousarray(w_qkv.T.reshape(3, 128, 1152))
    diag = np.zeros((9, 9, 128, 128), np.float32)
    idx = np.arange(128)
    for ti, (dy, dx) in enumerate(TAPS):
        for ct in range(9):
            diag[ti, ct, idx, idx] = w_dw[ct * 128 + idx, 0, dy + 1, dx + 1]
    wprojPT = np.ascontiguousarray(w_proj.T.reshape(4, 96, 384))
    tau = np.ones((128, 4), np.float32)
    p = np.arange(128)
    for dt in range(4):
        tau[:, dt] = temperature[np.minimum(2 * dt + (p >= 64), HEADS - 1)]
    acoefs = np.ascontiguousarray(np.broadcast_to(avals, (128, 4)).astype(np.float32))
    wc = np.zeros((9, 9, 128), np.float32)
    for ti, (dy, dx) in enumerate(TAPS):
        for ct in range(9):
            wc[ti, ct, :] = w_dw[ct * 128 + np.arange(128), 0, dy + 1, dx + 1]

    bf = ml_dtypes.bfloat16
    in_common = {
        "wqkvT": wqkvT.astype(np.float32),
        "diagw": diag.astype(np.float32),
        "wprojPT": wprojPT.astype(bf),
        "taucol": tau,
        "acoefs": acoefs,
        "wcols": wc,
    }
    in_maps = []
    for core in range(NCORES):
        xs = xpad[:, :, core * RPC: core * RPC + RPC + 2, :]
        xs = np.ascontiguousarray(xs.reshape(B, 3, 128, NPXH), np.float32)
        in_maps.append({"x_sh": xs, **in_common})
    return in_maps


def kernel(x, w_qkv, w_dw, w_proj, temperature, a1, a2, a3, a4):
    x = np.asarray(x, np.float32)
    w_qkv = np.asarray(w_qkv, np.float32)
    w_dw = np.asarray(w_dw, np.float32)
    w_proj = np.asarray(w_proj, np.float32)
    temperature = np.asarray(temperature, np.float32).reshape(HEADS)
    avals = np.array([float(np.asarray(a).reshape(())) for a in (a1, a2, a3, a4)],
                     np.float32)

    in_maps = _prep_inputs(x, w_qkv, w_dw, w_proj, temperature, avals)
    nc = _get_nc()
    res = bass_utils.run_bass_kernel_spmd(nc, in_maps, core_ids=list(range(NCORES)))

    out = np.empty((B, DIM, HH, WW), np.float32)
    for core in range(NCORES):
        o = res.results[core]["out_sh"].reshape(B, DIM, RPC, WW)
        out[:, :, core * RPC:(core + 1) * RPC, :] = o
    return out



# revision 8
# speedup vs baseline: 4.0515x; 4.0515x over previous
"""DRSformer sparse channel-attention block on 8 Trainium2 cores.

Sharding: the 128 image rows are split across 8 cores (16 rows each, 1-row
zero-padded halo). Per core: qkv 1x1-conv and the depthwise 3x3 conv run in
float32r (TF32-class) on TensorE — the depthwise conv as diagonal-stationary
matmuls PSUM-accumulated over the 9 taps on a 1-column-padded input so every
tap streams a flat 512-wide chunk; the two image-edge columns per row are
recomputed exactly on VectorE afterwards. q/k are split hi/lo into two bf16
planes (hi + residual) and DMA-transposed to [n, c] layout; per-head gram
matmuls (hi*hi + hi*lo + lo*hi) recover near-fp32 attention logits AND the
q/k l2-norms in one pass, contracting over the core's pixels. A 295KB
AllReduce combines partial grams across cores. Exact top-k selection uses a
rank matrix (all-pairs compare + row-sum); the four top-k softmaxes collapse
into one effective matrix P = E * sum_k (a_k/S_k)*[rank<=k], so all four
attention applications become a single P @ v matmul (bf16). Dense projection,
and the row-sharded output is gathered on host.
"""
import sys
for _p in ('/opt/trn_rl_repo', '/root/.axon_site/_ro/trn_rl_repo'):
    if _p not in sys.path:
        sys.path.insert(0, _p)

import numpy as np
import ml_dtypes

import concourse.bass as bass
import concourse.tile as tile
from concourse.tile import add_dep_helper
from concourse import mybir
from concourse import bass_utils
from concourse.masks import make_identity

f32 = mybir.dt.float32
f32r = mybir.dt.float32r
bf16 = mybir.dt.bfloat16
f16 = mybir.dt.float16
AF = mybir.ActivationFunctionType
OP = mybir.AluOpType

B, DIM, HEADS, HH, WW = 2, 384, 8, 128, 128
C = DIM // HEADS            # 48
NCORES = 8
RPC = HH // NCORES          # 16 rows per core
NPX = RPC * WW              # 2048 local pixels per batch
NPXH = (RPC + 2) * WW       # 2304 with halo rows
NCH = NPX // 128            # 16 n-chunks of 128
KVALS = [C // 2, C * 2 // 3, C * 3 // 4, C * 4 // 5]   # 24, 32, 36, 38
TAPS = [(0, 0), (-1, -1), (-1, 1), (1, -1), (1, 1), (0, -1), (0, 1), (-1, 0), (1, 0)]


def _ct_runs(h):
    """Head h's 48 channels as runs over 128-wide channel tiles:
    (ct, lo, n, c_off)."""
    out = []
    g0, c = h * C, 0
    while c < C:
        t, r = (g0 + c) // 128, (g0 + c) % 128
        n = min(C - c, 128 - r)
        out.append((t, r, n, c))
        c += n
    return out


def _build_bass():
    nc = bass.Bass("TRN2", target_bir_lowering=False, num_devices=NCORES)

    x_sh = nc.dram_tensor("x_sh", [B, 3, 128, NPXH], f16, kind="ExternalInput").ap()
    wqkvT = nc.dram_tensor("wqkvT", [3, 128, 1152], f16, kind="ExternalInput").ap()
    diagw = nc.dram_tensor("diagw", [9, 9, 128, 128], f32r, kind="ExternalInput").ap()
    wprojPT = nc.dram_tensor("wprojPT", [4, 96, 384], bf16, kind="ExternalInput").ap()
    taucol = nc.dram_tensor("taucol", [128, 4], f32, kind="ExternalInput").ap()
    acoefs = nc.dram_tensor("acoefs", [128, 4], f32, kind="ExternalInput").ap()
    wcols = nc.dram_tensor("wcols", [9, 9, 128], f32, kind="ExternalInput").ap()
    out_sh = nc.dram_tensor("out_sh", [B, 3, 128, NPX], f16, kind="ExternalOutput").ap()

    with tile.TileContext(nc) as tc:
        _build_body(nc, tc, x_sh, wqkvT, diagw, wprojPT, taucol, acoefs, wcols, out_sh)

    _split_excess_waits(nc)
    return nc


def _build_body(nc, tc, x_sh, wqkvT, diagw, wprojPT, taucol, acoefs, wcols, out_sh):
    import contextlib
    ctx = contextlib.ExitStack()
    consts = ctx.enter_context(tc.tile_pool(name="consts", bufs=1))
    xp = ctx.enter_context(tc.tile_pool(name="xp", bufs=1))      # 3 tags
    qkvp = ctx.enter_context(tc.tile_pool(name="qkvp", bufs=2))  # 1 tag
    cqp = ctx.enter_context(tc.tile_pool(name="cqp", bufs=1))    # 1 tag (hi/lo)
    cvp = ctx.enter_context(tc.tile_pool(name="cvp", bufs=1))    # 3 tags
    qkRp = ctx.enter_context(tc.tile_pool(name="qkRp", bufs=8))  # 1 tag
    qkTp = ctx.enter_context(tc.tile_pool(name="qkTp", bufs=4))  # 1 tag
    gramp = ctx.enter_context(tc.tile_pool(name="gramp", bufs=2))
    smallp = ctx.enter_context(tc.tile_pool(name="smallp", bufs=2))
    cmpp = ctx.enter_context(tc.tile_pool(name="cmpp", bufs=1))
    pTp = ctx.enter_context(tc.tile_pool(name="pTp", bufs=2))
    pvp = ctx.enter_context(tc.tile_pool(name="pvp", bufs=1))    # 4 tags
    outp = ctx.enter_context(tc.tile_pool(name="outp", bufs=2))
    dramp = ctx.enter_context(tc.tile_pool(name="dramp", bufs=2, space="DRAM"))
    psmm = ctx.enter_context(tc.tile_pool(name="psmm", bufs=4, space="PSUM"))
    psgram = ctx.enter_context(tc.tile_pool(name="psgram", bufs=2, space="PSUM"))
    pspT = ctx.enter_context(tc.tile_pool(name="pspT", bufs=2, space="PSUM"))

    # ---- constants ----
    wqkv_sb = consts.tile([128, 3, 1152], f16)
    nc.sync.dma_start(wqkv_sb, wqkvT.rearrange("k p o -> p k o"))
    diag_sb = consts.tile([128, 9, 9, 128], f32r)
    nc.sync.dma_start(diag_sb, diagw.rearrange("t c p f -> p t c f"))
    wproj_sb = consts.tile([96, 4, 384], bf16)
    nc.sync.dma_start(wproj_sb, wprojPT.rearrange("g p o -> p g o"))
    tau_sb = consts.tile([128, 4], f32)
    nc.sync.dma_start(tau_sb, taucol)
    ac_sb = consts.tile([128, 4], f32)
    nc.sync.dma_start(ac_sb, acoefs)
    wcol_sb = consts.tile([128, 9, 9], f32)
    nc.sync.dma_start(wcol_sb, wcols.rearrange("t c p -> p t c"))
    ident = consts.tile([128, 128], f32)
    make_identity(nc, ident)

    evict_flip = [0]
    last_evict = [None]

    def evict(dst, src):
        if evict_flip[0] % 2 == 0:
            e = nc.scalar.copy(dst, src)
        else:
            e = nc.vector.tensor_copy(dst, src)
        evict_flip[0] += 1
        last_evict[0] = e.ins
        return e

    prev_cc = [None]
    prev_gram_dma = [None]
    for b in range(B):
        # ---- load x ----
        x_t = []
        x_dma0 = [None]
        for kt in range(3):
            t = xp.tile([128, NPXH], f16, tag=f"x{kt}", name=f"x_{b}_{kt}")
            d = nc.sync.dma_start(t, x_sh[b, kt])
            if prev_cc[0] is not None:
                # order next batch's x loads after the previous batch's LAST
                # gram DMA (not the collective): avoids SP queue head-of-line
                # deadlock while letting b1 compute overlap b0's AllReduce
                add_dep_helper(d.ins, prev_gram_dma[0], reason="batch gate x")
            if x_dma0[0] is None:
                x_dma0[0] = d.ins
            x_t.append(t)

        def edge_chain(dst_col, x0, ct, qt):
            """Exact conv for an image-edge column (16 rows, stride 128)."""
            first = True
            for ti, (dy, dx) in enumerate(TAPS):
                if (x0 == 0 and dx < 0) or (x0 == 127 and dx > 0):
                    continue
                soff = 1 + (1 + dy) * 128 + x0 + dx
                sap = bass.AP(tensor=qt.tensor, offset=qt.offset + soff,
                              ap=[qt.ap[0], [128, RPC], [1, 1]])
                wc = wcol_sb[:, ti, ct:ct + 1]
                if first:
                    nc.vector.tensor_scalar(out=dst_col, in0=sap, scalar1=wc,
                                            scalar2=None, op0=OP.mult)
                    first = False
                else:
                    nc.vector.scalar_tensor_tensor(out=dst_col, in0=sap, scalar=wc,
                                                   in1=dst_col, op0=OP.mult, op1=OP.add)

        def qkv_conv(ct, hilo):
            """qkv projection + depthwise conv for one 128-channel tile.
            hilo=True: [128, 2, NPX] bf16 (hi plane + residual lo);
            else [128, NPX] bf16."""
            qt = qkvp.tile([128, NPXH + 2], f32r, tag="qkv", name=f"qkv_{b}_{ct}")
            m1 = nc.gpsimd.memset(qt[:, 0:1].bitcast(f32), 0.0)
            m2 = nc.gpsimd.memset(qt[:, NPXH + 1:NPXH + 2].bitcast(f32), 0.0)
            add_dep_helper(m1.ins, x_dma0[0], reason="batch gate qt pad")
            add_dep_helper(m2.ins, x_dma0[0], reason="batch gate qt pad")
            for ch0 in range(0, NPXH, 512):
                cw = min(512, NPXH - ch0)
                ps = psmm.tile([128, 512], f32, tag="mm", name="psq")
                for kt in range(3):
                    nc.tensor.matmul(
                        ps[:, :cw],
                        lhsT=wqkv_sb[:, kt, ct * 128:(ct + 1) * 128],
                        rhs=x_t[kt][:, ch0:ch0 + cw],
                        start=(kt == 0), stop=(kt == 2),
                    )
                evict(qt[:, 1 + ch0:1 + ch0 + cw], ps[:, :cw])
            if hilo:
                co = cqp.tile([128, 2, NPX], bf16, tag="cq", name=f"co_{b}_{ct}")
                hi_v = co[:, 0, :]
                lo_v = co[:, 1, :]
            else:
                co = cvp.tile([128, NPX], bf16, tag=f"cv{ct - 6}", name=f"co_{b}_{ct}")
                hi_v = co
                lo_v = None
            for ch in range(4):
                ps = psmm.tile([128, 512], f32, tag="mm", name="psc")
                for ti, (dy, dx) in enumerate(TAPS):
                    off = 129 + ch * 512 + dy * 128 + dx
                    nc.tensor.matmul(
                        ps, lhsT=diag_sb[:, ti, ct, :], rhs=qt[:, off:off + 512],
                        start=(ti == 0), stop=(ti == len(TAPS) - 1),
                    )
                sl = slice(ch * 512, (ch + 1) * 512)
                evict(hi_v[:, sl], ps)
                if hilo:
                    nc.vector.tensor_tensor(out=lo_v[:, sl], in0=ps,
                                            in1=hi_v[:, sl], op=OP.subtract)
            # exact edge-column fixup on the hi plane; zero the lo edges
            for x0 in (0, 127):
                hc = hi_v.rearrange("p (r w) -> p r w", w=128)[:, :, x0:x0 + 1]
                edge_chain(hc, x0, ct, qt)
                if hilo:
                    lc = lo_v.rearrange("p (r w) -> p r w", w=128)[:, :, x0:x0 + 1]
                    nc.vector.memset(lc, 0.0)
            return co

        # ---- q/k: qkv+conv -> hi/lo transpose -> per-head repack+gram ----
        # repack runs are issued per source raw tile so raws release early
        qkT_tiles = {}
        gram_dmas = []
        ar_in = dramp.tile([HEADS, 96, 96], f32, tag="arin", name=f"arin{b}")

        def get_qkT(h):
            if h not in qkT_tiles:
                qkT_tiles[h] = qkTp.tile([128, NCH, 4, 48], bf16, tag="qkT",
                                         name=f"qkT_{b}_{h}")
            return qkT_tiles[h]

        def gram(h):
            qkT = qkT_tiles[h]
            # region A (cols 0:96) accumulates hi.hi + lo.hi; region B
            # (96:192) accumulates hi.lo; summed at eviction. Folding hi.hi
            # and hi.lo into one FD=192 matmul halves PE dispatch count.
            gps = psgram.tile([96, 192], f32, tag="gram", name="gps")
            for t in range(NCH):
                hi = qkT[:, t, 0:2, :]
                lo = qkT[:, t, 2:4, :]
                both = qkT[:, t, :, :]
                if t < NCH - 1:
                    nc.tensor.matmul(gps, lhsT=hi, rhs=both,
                                     start=(t == 0), stop=False)
                    nc.tensor.matmul(gps[:, 0:96], lhsT=lo, rhs=hi,
                                     start=False, stop=False)
                else:
                    nc.tensor.matmul(gps[:, 0:96], lhsT=lo, rhs=hi,
                                     start=False, stop=False)
                    nc.tensor.matmul(gps, lhsT=hi, rhs=both,
                                     start=False, stop=True)
            gsb = gramp.tile([96, 96], f32, tag="gsb", name="gsb")
            evict(gsb, gps[:, 0:96])
            nc.vector.tensor_add(gsb, gsb, gps[:, 96:192])
            gd = nc.sync.dma_start(ar_in[h], gsb)
            gram_dmas.append(gd.ins)

        # HW-DGE completion under-synchronization: a consumer released by a
        # wide DmaTransposeAnt's first queue-completion can read data still
        # in flight on the DMA's other fanned-out queues. Work around it by
        # deferring each round's repack copies until the NEXT round's
        # transposes exist, and gating them on those (one full conv round of
        # slack), so the wide transposes have long drained before any read.
        pending = {r: [] for r in range(3)}      # round -> [(dst, src)]
        tr_insts = {r: [] for r in range(3)}

        def flush_round(rnd, gates):
            for dst, srcslice in pending[rnd]:
                e = evict(dst, srcslice)
                for g in gates:
                    add_dep_helper(e.ins, g, reason="transpose drain slack")
            pending[rnd].clear()
            for h in range(HEADS):
                if max(t for (t, _, _, _) in _ct_runs(h)) == rnd:
                    gram(h)

        for pair_ct in range(3):
            for qk in range(2):
                ct = qk * 3 + pair_ct
                co = qkv_conv(ct, hilo=True)
                for pl in range(2):
                    tr = qkRp.tile([128, NCH, 128], bf16, tag="qkr",
                                   name=f"qkr_{b}_{ct}_{pl}")
                    # transposes isolated on the Activation DGE queues:
                    # concurrent plain copies on the same queues corrupt
                    # xbar-mode transposes (known HW hazard, untracked here)
                    td = nc.scalar.dma_start_transpose(tr, co[:, pl, :])
                    tr_insts[pair_ct].append(td.ins)
                    # planes in qkT: [q_hi | k_hi | q_lo | k_lo]
                    for h in range(HEADS):
                        for (t, r, n, c) in _ct_runs(h):
                            if t == pair_ct:
                                pending[pair_ct].append(
                                    (get_qkT(h)[:, :, 2 * pl + qk, c:c + n],
                                     tr[:, :, r:r + n]))
            if pair_ct > 0:
                flush_round(pair_ct - 1, tr_insts[pair_ct])

        # ---- v ----
        cv_t = []
        for ct in range(6, 9):
            cv_t.append(qkv_conv(ct, hilo=False))
        flush_round(2, [last_evict[0]])

        # ---- AllReduce partial grams ----
        ar_out = dramp.tile([HEADS, 96, 96], f32, tag="arout", name=f"arout{b}")
        cc = nc.gpsimd.collective_compute(
            "AllReduce", OP.add,
            replica_groups=[list(range(NCORES))],
            ins=[ar_in[:].opt()], outs=[ar_out[:].opt()],
        )
        for gd in gram_dmas:
            # explicit sem deps: the collective must not read ar_in before
            # every gram DMA has landed (Tile's transitive-clock reasoning
            # proved unsound for this on HW)
            add_dep_helper(cc.ins, gd, reason="cc waits gram dmas")
        prev_cc[0] = cc.ins
        prev_gram_dma[0] = gram_dmas[-1]

        # ---- post-AllReduce: dense tiles, 2 heads per tile at 64-row pitch ----
        arf = ar_out.rearrange("h i j -> (h i j)")
        kdiag = smallp.tile([HEADS, 48], f32, tag="kdiag", name="kdiag")
        for h in range(HEADS):
            base = h * 96 * 96 + 48 * 96 + 48
            src = bass.AP(tensor=arf.tensor, offset=arf.offset + base,
                          ap=[[0, 1], [97, 48]])
            _d = nc.sync.dma_start(kdiag[h:h + 1, :], src)
            add_dep_helper(_d.ins, cc.ins, reason="post-AR read after cc")
        kdd = dramp.tile([HEADS, 48], f32, tag="kdd", name=f"kdd{b}")
        nc.sync.dma_start(kdd, kdiag)

        pv_t = []
        for dt in range(4):
            at = smallp.tile([128, 48], f32, tag="attn", name="at")
            rq = smallp.tile([128, 1], f32, tag="rq", name="rq")
            rk = smallp.tile([128, 48], f32, tag="rk", name="rk")
            for _t in (at, rq, rk):
                _m = nc.gpsimd.memset(_t, 1.0)
                add_dep_helper(_m.ins, prev_cc[0], reason="post-AR gate")
            for e in range(2):
                h = 2 * dt + e
                r = 64 * e
                base = h * 96 * 96
                src = bass.AP(tensor=arf.tensor, offset=arf.offset + base + 48,
                              ap=[[96, 48], [1, 48]])
                _d1 = nc.sync.dma_start(at[r:r + 48, :], src)
                add_dep_helper(_d1.ins, cc.ins, reason="post-AR read after cc")
                srcq = bass.AP(tensor=arf.tensor, offset=arf.offset + base,
                               ap=[[97, 48], [1, 1]])
                _d2 = nc.sync.dma_start(rq[r:r + 48, :], srcq)
                add_dep_helper(_d2.ins, cc.ins, reason="post-AR read after cc")
                nc.sync.dma_start(rk[r:r + 48, :],
                                  kdd[h:h + 1, :].broadcast_to((48, 48)))

            # ---- normalize, rank, blended masked softmax ----
            nc.vector.reciprocal(rq, rq)
            nc.scalar.sqrt(rq, rq)
            nc.vector.reciprocal(rk, rk)
            nc.scalar.sqrt(rk, rk)
            an = smallp.tile([128, 48], f32, tag="an", name="an")
            nc.vector.tensor_scalar(out=an, in0=at, scalar1=rq,
                                    scalar2=None, op0=OP.mult)
            nc.vector.tensor_mul(an, an, rk)
            rank = smallp.tile([128, 48], f32, tag="rank", name="rank")
            for half in range(2):
                cmp = cmpp.tile([128, 24, 48], bf16, tag="cmp", name="cmp")
                io = half * 24
                in_j = bass.AP(tensor=an.tensor, offset=an.offset,
                               ap=[an.ap[0], [0, 24], [1, 48]])
                in_i = bass.AP(tensor=an.tensor, offset=an.offset + io,
                               ap=[an.ap[0], [1, 24], [0, 48]])
                nc.vector.tensor_tensor(out=cmp, in0=in_j, in1=in_i, op=OP.is_ge)
                nc.vector.tensor_reduce(out=rank[:, io:io + 24], in_=cmp,
                                        axis=mybir.AxisListType.X, op=OP.add)
            E = smallp.tile([128, 48], f32, tag="E", name="E")
            nc.scalar.activation(E, an, AF.Exp, scale=tau_sb[:, dt:dt + 1])
            W = smallp.tile([128, 48], f32, tag="W", name="W")
            junk = smallp.tile([128, 48], f32, tag="junk", name="junk")
            S = smallp.tile([128, 1], f32, tag="S", name="S")
            wcolv = smallp.tile([128, 1], f32, tag="wcolv", name="wcolv")
            for ki, kk in enumerate(KVALS):
                mk = smallp.tile([128, 48], bf16, tag="mk", name="mk")
                nc.vector.tensor_scalar(out=mk, in0=rank, scalar1=float(kk),
                                        scalar2=None, op0=OP.is_le)
                nc.vector.tensor_mul(junk, E, mk)
                nc.vector.tensor_reduce(out=S, in_=junk,
                                        axis=mybir.AxisListType.X, op=OP.add)
                nc.vector.reciprocal(S, S)
                nc.vector.tensor_mul(wcolv, S, ac_sb[:, ki:ki + 1])
                if ki == 0:
                    nc.vector.tensor_scalar(out=W, in0=mk, scalar1=wcolv,
                                            scalar2=None, op0=OP.mult)
                else:
                    nc.vector.scalar_tensor_tensor(out=W, in0=mk, scalar=wcolv,
                                                   in1=W, op0=OP.mult, op1=OP.add)
            P = smallp.tile([128, 48], f32, tag="P", name="P")
            nc.vector.tensor_mul(P, E, W)

            # ---- P^T pieces into v-aligned pair stationaries ----
            pair = dt
            pT = {}
            for e in range(2):
                for (vt, k0, nd, d0) in _ct_runs(2 * pair + e):
                    if (pair, vt) not in pT:
                        t = pTp.tile([128, 96], bf16, tag="pT", name=f"pT{pair}_{vt}")
                        _m = nc.vector.memset(t, 0.0)
                        add_dep_helper(_m.ins, prev_cc[0], reason="post-AR gate")
                        pT[(pair, vt)] = t
            for e in range(2):
                h = 2 * pair + e
                r = 64 * e
                tps = pspT.tile([48, 48], f32, tag="tps", name="tps")
                nc.tensor.transpose(tps, P[r:r + 48, :], ident[r:r + 48, r:r + 48])
                piece = smallp.tile([48, 48], bf16, tag="piece", name="piece")
                evict(piece, tps)
                for (vt, k0, nd, d0) in _ct_runs(h):
                    nc.sync.dma_start(
                        pT[(pair, vt)][k0:k0 + nd, e * 48: e * 48 + 48],
                        piece[d0:d0 + nd, :])

            # ---- P @ v for this pair ----
            pvt = pvp.tile([96, NPX], bf16, tag=f"pv{pair}", name=f"pv_{b}_{pair}")
            vts = sorted({vt for e in range(2)
                          for (vt, _, _, _) in _ct_runs(2 * pair + e)})
            for ch in range(4):
                ps = psmm.tile([128, 512], f32, tag="mm", name="pspv")
                for vi, vt in enumerate(vts):
                    nc.tensor.matmul(ps[:96, :], lhsT=pT[(pair, vt)],
                                     rhs=cv_t[vt][:, ch * 512:(ch + 1) * 512],
                                     start=(vi == 0), stop=(vi == len(vts) - 1))
                evict(pvt[:, ch * 512:(ch + 1) * 512], ps[:96, :])
            pv_t.append(pvt)

        # ---- out = Wproj @ pv ----
        for ot in range(3):
            for ch in range(4):
                ps = psmm.tile([128, 512], f32, tag="mm", name="pso")
                for p in range(4):
                    nc.tensor.matmul(ps, lhsT=wproj_sb[:, p, ot * 128:(ot + 1) * 128],
                                     rhs=pv_t[p][:, ch * 512:(ch + 1) * 512],
                                     start=(p == 0), stop=(p == 3))
                ot_sb = outp.tile([128, 512], f16, tag="osb", name="osb")
                evict(ot_sb, ps)
                nc.sync.dma_start(out_sh[b, ot, :, ch * 512:(ch + 1) * 512], ot_sb)

    ctx.close()


def _split_excess_waits(nc, cap=1):
    """walrus allows 1 sync-wait per instruction; Tile's tail drain can carry
    more — split extras into single-wait drains."""
    n_new = 0
    for fn in nc.m.functions:
        for bb in fn.blocks:
            insts = bb.instructions
            i = 0
            while i < len(insts):
                inst = insts[i]
                si = inst.sync_info
                if si is not None and len(si.on_wait) > cap:
                    waits = list(si.on_wait)
                    extras, keep = waits[:-cap], waits[-cap:]
                    inst.sync_info = mybir.SyncInfo(on_wait=keep,
                                                    on_update=list(si.on_update))
                    for w in extras:
                        d = mybir.InstDrain(name=f"{inst.name}-sw{n_new}",
                                            ins=[], outs=[])
                        d.engine = inst.engine
                        d.sync_info = mybir.SyncInfo(on_wait=[w], on_update=[])
                        nc.register_instruction(d, overwrite=True)
                        insts.insert(i, d)
                        i += 1
                        n_new += 1
                i += 1
    return n_new


_NC_CACHE = {}


def _get_nc():
    if "nc" not in _NC_CACHE:
        _NC_CACHE["nc"] = _build_bass()
    return _NC_CACHE["nc"]


def _prep_weights(w_qkv, w_dw, w_proj, temperature, avals):
    """Per-core weight arrays (identical on every core)."""
    wqkvT = np.ascontiguousarray(w_qkv.T.reshape(3, 128, 1152))
    diag = np.zeros((9, 9, 128, 128), np.float32)
    idx = np.arange(128)
    for ti, (dy, dx) in enumerate(TAPS):
        for ct in range(9):
            diag[ti, ct, idx, idx] = w_dw[ct * 128 + idx, 0, dy + 1, dx + 1]
    wprojPT = np.ascontiguousarray(w_proj.T.reshape(4, 96, 384))
    tau = np.ones((128, 4), np.float32)
    p = np.arange(128)
    for dt in range(4):
        tau[:, dt] = temperature[np.minimum(2 * dt + (p >= 64), HEADS - 1)]
    acoefs = np.ascontiguousarray(np.broadcast_to(avals, (128, 4)).astype(np.float32))
    wc = np.zeros((9, 9, 128), np.float32)
    for ti, (dy, dx) in enumerate(TAPS):
        for ct in range(9):
            wc[ti, ct, :] = w_dw[ct * 128 + np.arange(128), 0, dy + 1, dx + 1]

    bf = ml_dtypes.bfloat16
    return {
        "wqkvT": wqkvT.astype(np.float16),
        "diagw": diag.astype(np.float32),
        "wprojPT": wprojPT.astype(bf),
        "taucol": tau,
        "acoefs": acoefs,
        "wcols": wc,
    }


def _prep_x(x):
    """Global sharded-x array [NCORES*B, 3, 128, NPXH] fp16 (core-major),
    rows split 16-per-core with a 1-row zero halo."""
    xpad = np.zeros((B, 3, 128, HH + 2, WW), np.float16)
    xpad[:, :, :, 1:HH + 1] = x.reshape(B, 3, 128, HH, WW)
    xg = np.empty((NCORES, B, 3, 128, NPXH), np.float16)
    for core in range(NCORES):
        xg[core] = xpad[:, :, :, core * RPC: core * RPC + RPC + 2].reshape(
            B, 3, 128, NPXH)
    return xg.reshape(NCORES * B, 3, 128, NPXH)


def _get_runner():
    """Build (once) the cached jitted SPMD callable and device mesh.

    Mirrors bass2jax.run_bass_via_pjrt, minus its per-call overheads: the
    jit object is reused across calls (no re-trace), weights stay
    device-resident, and no donated zero output buffers are shipped — the
    kernel writes every out_sh element, so PJRT's uninitialized custom-call
    result buffers are fine.
    """
    if "runner" in _NC_CACHE:
        return _NC_CACHE["runner"]
    import jax
    from jax.experimental.shard_map import shard_map
    from jax.sharding import Mesh, PartitionSpec, NamedSharding
    from concourse.bass2jax import (
        _bass_exec_p, partition_id_tensor, install_neuronx_cc_hook)

    nc = _get_nc()
    install_neuronx_cc_hook()
    assert not nc.dbg_callbacks if nc.dbg_addr is not None else True

    partition_name = nc.partition_id_tensor.name if nc.partition_id_tensor else None
    in_names, out_names, out_avals = [], [], []
    for alloc in nc.m.functions[0].allocations:
        if not isinstance(alloc, mybir.MemoryLocationSet):
            continue
        name = alloc.memorylocations[0].name
        if alloc.kind == "ExternalInput":
            if name != partition_name:
                in_names.append(name)
        elif alloc.kind == "ExternalOutput":
            out_names.append(name)
            out_avals.append(jax.core.ShapedArray(
                tuple(alloc.tensor_shape), mybir.dt.np(alloc.dtype)))
    bind_names = list(in_names)
    if partition_name is not None:
        bind_names.append(partition_name)

    def _body(*args):
        operands = list(args)
        if partition_name is not None:
            operands.append(partition_id_tensor())
        outs = _bass_exec_p.bind(
            *operands,
            out_avals=tuple(out_avals),
            in_names=tuple(bind_names),
            out_names=tuple(out_names),
            lowering_input_output_aliases=(),
            sim_require_finite=True,
            sim_require_nnan=True,
            nc=nc,
        )
        return tuple(outs)

    devices = jax.devices()[:NCORES]
    assert len(devices) == NCORES
    mesh = Mesh(np.asarray(devices), ("core",))
    sharded = jax.jit(
        shard_map(_body, mesh=mesh,
                  in_specs=(PartitionSpec("core"),) * len(in_names),
                  out_specs=(PartitionSpec("core"),) * len(out_names),
                  check_rep=False),
        keep_unused=True,
    )
    runner = {
        "sharded": sharded,
        "in_names": in_names,
        "sharding": NamedSharding(mesh, PartitionSpec("core")),
        "jax": jax,
        "host_w": None,     # host copies for change detection
        "dev_w": None,      # name -> device-resident global array
        "dbg_name": nc.dbg_addr.name if nc.dbg_addr is not None else None,
    }
    _NC_CACHE["runner"] = runner
    return runner


def _ensure_weights(runner, wmap):
    """Upload weight arrays to the device mesh once; re-upload only if the
    values change between kernel() calls."""
    jax = runner["jax"]
    if runner["host_w"] is not None and all(
            np.array_equal(runner["host_w"][k], v) for k, v in wmap.items()):
        return
    full = dict(wmap)
    if runner["dbg_name"] is not None:
        full[runner["dbg_name"]] = np.zeros((1, 2), np.uint32)
    dev_w = {}
    for name, a in full.items():
        g = np.concatenate([a] * NCORES, axis=0)
        dev_w[name] = jax.device_put(g, runner["sharding"])
    for v in dev_w.values():
        v.block_until_ready()
    runner["host_w"] = {k: v.copy() for k, v in wmap.items()}
    runner["dev_w"] = dev_w


def _invoke(xg):
    """One timed device invocation: ship sharded x (fp16), run the cached
    executable, fetch the full fp16 output [NCORES*B, 3, 128, NPX]."""
    runner = _get_runner()
    jax = runner["jax"]
    xd = jax.device_put(xg, runner["sharding"])
    args = [xd if name == "x_sh" else runner["dev_w"][name]
            for name in runner["in_names"]]
    out = runner["sharded"](*args)
    return np.asarray(out[0])


def _assemble(res):
    """[NCORES*B, 3, 128, NPX] fp16 -> full [B, DIM, HH, WW] fp32."""
    o = res.reshape(NCORES, B, 3, 128, RPC, WW).astype(np.float32)
    # out[b, kt*128+p, core*RPC+r, w] = o[core, b, kt, p, r, w]
    return np.ascontiguousarray(
        o.transpose(1, 2, 3, 0, 4, 5)).reshape(B, DIM, HH, WW)


def kernel(x, w_qkv, w_dw, w_proj, temperature, a1, a2, a3, a4):
    x = np.asarray(x, np.float32)
    w_qkv = np.asarray(w_qkv, np.float32)
    w_dw = np.asarray(w_dw, np.float32)
    w_proj = np.asarray(w_proj, np.float32)
    temperature = np.asarray(temperature, np.float32).reshape(HEADS)
    avals = np.array([float(np.asarray(a).reshape(())) for a in (a1, a2, a3, a4)],
                     np.float32)

    runner = _get_runner()
    _ensure_weights(runner, _prep_weights(w_qkv, w_dw, w_proj, temperature, avals))
    res = _invoke(_prep_x(x))
    return _assemble(res)

